# revision 60
# baseline (speedup 1.0000x reference)
"""BertBlock (mean-only LN, 16-head attention, relu FF) on 8 trn2 NeuronCores.

Sharding: head-parallel attention (2 heads / core) + sequence-parallel FF
(512 rows / core, scattered in 4x128-row blocks to match ReduceScatter chunk
placement). No AllGather: every core reads the full transposed input xT
(bf16) and computes LN1 means locally on the PE (mean folded into the QKV
matmul as a rank-1 correction; LN scale/bias folded into weights on host).
One bf16 ReduceScatter after out-proj, split into 4 row-chunks overlapped
under attention compute. FF runs locally with streamed bf16 weights.
"""
import sys

sys.path.insert(0, '/opt/trn_rl_repo')

import numpy as np
import ml_dtypes
import concourse.bass as bass
from concourse import bacc
import concourse.mybir as mybir
import concourse.tile as tile
from concourse.masks import make_identity

S = 4096          # sequence length
H = 1024          # hidden
I_ = 4096         # ffn inner
NH = 16           # heads
HD = 64           # head dim
NC = 8            # cores
SM = S // NC      # 512 rows per core
DM = 128          # inner dims per core (2 heads x 64)
HC = H // 128     # 8 hidden chunks
ST = S // 512     # 8 s-tiles of 512
RQ = 4            # ReduceScatter chunks
F32 = mybir.dt.float32
F32R = mybir.dt.float32r
BF16 = mybir.dt.bfloat16
AF = mybir.ActivationFunctionType
ALU = mybir.AluOpType
AXX = mybir.AxisListType.X

_CACHE = {}
DEBUG = False


def build_nc():
    nc = bacc.Bacc(None, target_bir_lowering=False, debug=False)
    P = lambda name, shape, dt=F32: nc.declare_dram_parameter(name, shape, dt,
                                                              isOutput=False)
    xT = P("xT", [H, S], BF16)           # full normed-input-free transposed x
    wqkvT = P("wqkvT", [H, 3 * DM], BF16)  # anw-folded [h, q|k|v of my 2 heads]
    nrw = P("nrw", [1, 3 * DM])          # -colsums of wqkvT (mean correction)
    bqkv = P("bqkv", [1, 3 * DM])        # anb-folded biases
    owT = P("owT", [DM, H], BF16)        # o_w[:, my_cols].T
    ob = P("ob", [1, H])
    ff1wT = P("ff1wT", [H, I_], BF16)    # fnw-folded
    ff1b = P("ff1b", [32, 128])          # fnb-folded
    ff2wT = P("ff2wT", [I_, H], BF16)
    ffb2 = P("ffb2", [1, H])
    x_res = P("x_res", [SM, H])          # my owned rows (4 scattered 128-blocks)
    y = nc.declare_dram_parameter("y", [SM, H], F32, isOutput=True)
    if DEBUG:
        dbg_qt = nc.declare_dram_parameter("dbg_qt", [128, S], BF16, isOutput=True)
        dbg_kt = nc.declare_dram_parameter("dbg_kt", [128, S], BF16, isOutput=True)
        dbg_ctx = nc.declare_dram_parameter("dbg_ctx", [128, S], BF16, isOutput=True)
        dbg_rs = nc.declare_dram_parameter("dbg_rs", [SM, H], BF16, isOutput=True)

    with tile.TileContext(nc) as tc:
        cst = tc.alloc_tile_pool(name="cst", bufs=1)
        dram = tc.alloc_tile_pool(name="dram", bufs=1, space="DRAM")
        setp = tc.alloc_tile_pool(name="setp", bufs=1)
        ps_set = tc.alloc_tile_pool(name="ps_set", bufs=2, space="PSUM")

        rs_in = [dram.tile([1024, H], BF16, tag=f"rsi{q}", name=f"rsi{q}")
                 for q in range(RQ)]
        rs_out = [dram.tile([128, H], BF16, tag=f"rso{q}", name=f"rso{q}")
                  for q in range(RQ)]

        # ---- constants ----
        ident_f = cst.tile([128, 128], F32)
        make_identity(nc, ident_f)
        ones_f = cst.tile([1, 128], F32)
        nc.gpsimd.memset(ones_f, 1.0)
        ones1 = cst.tile([1, 128], F32R)
        nc.vector.tensor_copy(ones1[:], ones_f[:])
        ones_col_b = cst.tile([128, 1], BF16)
        nc.gpsimd.memset(ones_col_b, 1.0)

        def load_vec(p):
            t = setp.tile([1, H], F32, tag=f"v_{p.name}")
            nc.sync.dma_start(out=t[:], in_=p[:])
            return t

        vecs = {n: load_vec(p) for n, p in [("ob", ob), ("ffb2", ffb2)]}

        def bcast(name):
            # [1, H] -> [128, H] broadcast across partitions via PE
            v = vecs[name]
            bc = cst.tile([128, H], F32, tag=f"bc_{name}", name=f"bc_{name}")
            for hf in range(H // 512):
                ps = ps_set.tile([128, 512], F32)
                nc.tensor.matmul(ps[:], ones_f[0:1, :], v[0:1, hf * 512:(hf + 1) * 512],
                                 start=True, stop=True)
                nc.vector.tensor_copy(bc[:, hf * 512:(hf + 1) * 512], ps[:])
            return bc

        ob_bc, ffb2_bc = bcast("ob"), bcast("ffb2")

        # qkv biases as per-partition columns [128, 1] x3
        bqkv_sb = setp.tile([1, 3 * DM], F32)
        nc.sync.dma_start(out=bqkv_sb[:], in_=bqkv[:])
        qkvb_pp = []
        for j in range(3):
            ps = ps_set.tile([128, 512], F32)
            nc.tensor.matmul(ps[:, 0:1],
                             bqkv_sb[0:1, j * 128:(j + 1) * 128],
                             ones_f[0:1, 0:1], start=True, stop=True)
            t = cst.tile([128, 1], F32, tag=f"b_pp{j}")
            nc.vector.tensor_copy(t[:], ps[:, 0:1])
            qkvb_pp.append(t)

        # neg row-sums of wqkv as f32r rows for the mean-correction matmul
        nrw_ld = setp.tile([1, 3 * DM], F32)
        nc.sync.dma_start(out=nrw_ld[:], in_=nrw[:])
        nrw_r = cst.tile([1, 3 * DM], F32R)
        nc.vector.tensor_copy(nrw_r[:], nrw_ld[:])

        # ff1 bias transposed to per-partition layout [128, 32]
        ffb1_ld = setp.tile([32, 128], F32)
        nc.sync.dma_start(out=ffb1_ld[:], in_=ff1b[:])
        ps = ps_set.tile([128, 512], F32)
        nc.tensor.transpose(ps[:, 0:32], ffb1_ld[:], ident_f[0:32, 0:32])
        ffb1_pp = cst.tile([128, 32], F32)
        nc.vector.tensor_copy(ffb1_pp[:], ps[:, 0:32])
        ps_set.release()
        setp.release()

        # ---- tiles shared by phases B and C (x2 / xn2T built per RS chunk
        # inside phase B so the FF input is ready when attention ends) ----
        ffpre = tc.alloc_tile_pool(name="ffpre", bufs=1)
        x2_tiles = [ffpre.tile([128, H], F32, tag=f"x2{i}", name=f"x2{i}")
                    for i in range(4)]
        xn2T = [ffpre.tile([128, SM], BF16, tag=f"xn2T{hc}", name=f"xn2T{hc}")
                for hc in range(HC)]
        xres_tiles = [ffpre.tile([128, H], F32, tag=f"xres{i}",
                                 name=f"xres{i}") for i in range(4)]

        # first half of ff1 weights, preloaded during attention (DMA idle)
        w1pre = tc.alloc_tile_pool(name="w1pre", bufs=1)
        w1pre_t = [[w1pre.tile([128, 512], BF16, tag=f"w1e{ib}_{hc}",
                               name=f"w1e{ib}_{hc}")
                    for hc in range(HC)] for ib in range(4)]
        w2pre_t = [w1pre.tile([128, 512], BF16, tag=f"w2e{ic}",
                              name=f"w2e{ic}") for ic in range(16)]

        # ---- attention state tiles ----
        at = tc.alloc_tile_pool(name="at", bufs=1)
        wqkv_t = []
        for hc in range(HC):
            t = at.tile([128, 3 * DM], BF16, tag=f"wqkv{hc}")
            nc.sync.dma_start(out=t[:], in_=wqkvT[hc * 128:(hc + 1) * 128, :])
            wqkv_t.append(t)
        owT_sb = at.tile([DM, H], BF16, tag="owT")
        nc.sync.dma_start(out=owT_sb[:], in_=owT[:])

        QTp = [at.tile([128, S], BF16, tag=f"QTp{h}", name=f"QTp{h}")
               for h in range(2)]
        KTp = [at.tile([128, S], BF16, tag=f"KTp{h}", name=f"KTp{h}")
               for h in range(2)]
        for h in range(2):
            z = slice(HD, 128) if h == 0 else slice(0, HD)
            nc.gpsimd.memset(QTp[h][z, :], 0.0)
            nc.gpsimd.memset(KTp[h][z, :], 0.0)
        # V+ones per t-PAIR in fp8e4, k-subtile layout [128, 2, 66] so the
        # ctx accumulation runs in DoubleRow mode (256-contract, 0.5 c/row)
        F8 = mybir.dt.float8e4
        vaug = [[at.tile([128, 2, 128], F8, tag=f"va{h}_{tp}",
                         name=f"va{h}_{tp}") for tp in range(16)]
                for h in range(2)]
        for h in range(2):
            for tp in range(16):
                nc.gpsimd.memset(vaug[h][tp][:], 0.0)
                for u in range(2):
                    nc.gpsimd.memset(vaug[h][tp][:, u, HD:HD + 1], 1.0)
        ctxT = at.tile([128, S], BF16, tag="ctxT")

        # ---- phase A: stream xT, compute means on PE, QKV with rank-1
        # mean correction folded into the matmul ----
        with tc.tile_pool(name="xtp", bufs=2) as xtp, \
             tc.tile_pool(name="musb", bufs=3) as musb, \
             tc.tile_pool(name="vtp", bufs=3) as vtp, \
             tc.tile_pool(name="ps_mu", bufs=2, space="PSUM") as ps_mu, \
             tc.tile_pool(name="ps_qkv", bufs=3, space="PSUM") as ps_qkv, \
             tc.tile_pool(name="ps_vt", bufs=2, space="PSUM") as ps_vt:
            for r in range(ST):
                xtr = []
                for hc in range(HC):
                    t = xtp.tile([128, 512], BF16, tag=f"xtr{hc}")
                    nc.sync.dma_start(
                        out=t[:], in_=xT[hc * 128:(hc + 1) * 128,
                                         r * 512:(r + 1) * 512])
                    xtr.append(t)
                pmu = ps_mu.tile([1, 512], F32, tag="mu")
                for hc in range(HC):
                    nc.tensor.matmul(pmu[:], ones_col_b[:], xtr[hc][:],
                                     start=(hc == 0), stop=(hc == 7))
                mu = musb.tile([1, 512], F32R, tag="mu_sb")
                nc.vector.tensor_copy(mu[:], pmu[:])
                for j in range(3):
                    psq = ps_qkv.tile([128, 512], F32, tag="qkv")
                    for hc in range(HC):
                        nc.tensor.matmul(psq[:],
                                         wqkv_t[hc][:, j * 128:(j + 1) * 128],
                                         xtr[hc][:], start=(hc == 0), stop=False)
                    # += (-rowsum_j/H) outer mu  (removes the mean term)
                    nc.tensor.matmul(psq[:], nrw_r[0:1, j * 128:(j + 1) * 128],
                                     mu[:], start=False, stop=True)
                    if j < 2:
                        dest = QTp if j == 0 else KTp
                        for h in range(2):
                            hs = slice(h * HD, (h + 1) * HD)
                            nc.scalar.activation(
                                dest[h][hs, r * 512:(r + 1) * 512], psq[hs, :],
                                AF.Identity, bias=qkvb_pp[j][hs, :])
                    else:
                        vtmp = vtp.tile([128, 512], F32, tag="vtmp")
                        nc.scalar.activation(vtmp[:], psq[:], AF.Identity,
                                             bias=qkvb_pp[2][:])
                        for tb in range(4):
                            pst = ps_vt.tile([128, 128], F32, tag="vt")
                            nc.tensor.transpose(
                                pst[:], vtmp[:, tb * 128:(tb + 1) * 128],
                                ident_f[:])
                            ti = r * 4 + tb
                            tp, u = ti // 2, ti % 2
                            nc.vector.tensor_copy(vaug[0][tp][:, u, 0:HD],
                                                  pst[:, 0:HD])
                            nc.vector.tensor_copy(vaug[1][tp][:, u, 0:HD],
                                                  pst[:, HD:2 * HD])

        for ib in range(4):
            for hc in range(HC):
                nc.sync.dma_start(
                    out=w1pre_t[ib][hc][:],
                    in_=ff1wT[hc * 128:(hc + 1) * 128,
                              ib * 512:(ib + 1) * 512])

        # ---- phase B: attention (h-paired chains, out-proj + chunked RS
        # interleaved) ----
        with tc.tile_pool(name="expp", bufs=6) as expp, \
             tc.tile_pool(name="tip", bufs=3) as tip, \
             tc.tile_pool(name="rcp", bufs=4) as rcp, \
             tc.tile_pool(name="aop", bufs=3) as aop, \
             tc.tile_pool(name="ps_sc", bufs=4, space="PSUM") as ps_sc, \
             tc.tile_pool(name="ps_cx", bufs=1, space="PSUM") as ps_cx, \
             tc.tile_pool(name="ps_o", bufs=1, space="PSUM") as ps_o:
            for r in range(ST):
                cps_l = [ps_cx.tile([128, 512], F32, name=f"cps{r}_{h}",
                                    tag=f"cps{h}") for h in range(2)]
                for tp in range(16):
                    exl = []
                    for h in range(2):
                        ex2 = expp.tile([128, 2, 512], F8, tag="exp")
                        for u in range(2):
                            t = 2 * tp + u
                            sps = ps_sc.tile([128, 512], F32, tag="sc")
                            nc.tensor.matmul(
                                sps[:], KTp[h][:, t * 128:(t + 1) * 128],
                                QTp[h][:, r * 512:(r + 1) * 512],
                                start=True, stop=True)
                            if (2 * t + h) % 9 in (2, 6):
                                # Schraudolph exp on DVE: float bits of
                                # int(x*2^23/ln2*0.125 + B) approximate
                                # exp(x*0.125) to ~2-3%; offloads Act
                                ti = tip.tile([128, 512], mybir.dt.int32,
                                              tag="ti")
                                nc.vector.tensor_scalar(
                                    out=ti[:], in0=sps[:],
                                    scalar1=1512775.3952,
                                    scalar2=1064866805.0,
                                    op0=ALU.mult, op1=ALU.add)
                                nc.vector.tensor_copy(ex2[:, u, :],
                                                      ti[:].bitcast(F32))
                            else:
                                nc.scalar.activation(ex2[:, u, :], sps[:],
                                                     AF.Exp, scale=0.125)
                        exl.append(ex2)
                    for h, ex2 in enumerate(exl):
                        nc.tensor.matmul(cps_l[h][0:128, :],
                                         vaug[h][tp][:, :, :], ex2[:, :, :],
                                         start=(tp == 0), stop=(tp == 15),
                                         perf_mode=mybir.MatmulPerfMode.DoubleRow,
                                         skip_group_check=True)
                for h in range(2):
                    hs = slice(h * HD, (h + 1) * HD)
                    den = rcp.tile([1, 512], F32, tag="den")
                    nc.vector.tensor_copy(den[:], cps_l[h][HD:HD + 1, :])
                    rc = rcp.tile([1, 512], F32, tag="rc")
                    nc.vector.reciprocal_approx_fast(out=rc[:], in_=den[:])
                    rc_r = rcp.tile([1, 512], F32R, tag="rc_r")
                    nc.vector.tensor_copy(rc_r[:], rc[:])
                    bps = ps_o.tile([128, 512], F32, tag="rbc")
                    nc.tensor.matmul(bps[0:HD, :], ones1[0:1, 0:HD],
                                     rc_r[:], start=True, stop=True)
                    bsb = rcp.tile([HD, 512], F32, tag="bsb")
                    nc.vector.tensor_copy(bsb[:], bps[0:HD, :])
                    nc.vector.tensor_mul(ctxT[hs, r * 512:(r + 1) * 512],
                                         cps_l[h][0:HD, :], bsb[:])
                for s128 in range(4 * r, 4 * r + 4):
                    ao = aop.tile([128, H], BF16, tag="ao")
                    for hf in range(2):
                        ps = ps_o.tile([128, 512], F32, tag="op")
                        nc.tensor.matmul(ps[:],
                                         ctxT[:, s128 * 128:(s128 + 1) * 128],
                                         owT_sb[:, hf * 512:(hf + 1) * 512],
                                         start=True, stop=True)
                        nc.vector.tensor_copy(ao[:, hf * 512:(hf + 1) * 512],
                                              ps[:])
                    nc.sync.dma_start(
                        out=rs_in[s128 // 8][(s128 % 8) * 128:
                                             (s128 % 8 + 1) * 128, :],
                        in_=ao[:])
                if r % 2 == 1:
                    q = r // 2
                    nc.gpsimd.collective_compute(
                        "ReduceScatter", ALU.add,
                        replica_groups=[list(range(NC))],
                        ins=[rs_in[q].opt()],
                        outs=[rs_out[q].opt()])
                if r == 3:
                    for i in range(4):
                        nc.sync.dma_start(
                            out=xres_tiles[i][:],
                            in_=x_res[i * 128:(i + 1) * 128, :])
                if r == 5:
                    for ic in range(16):
                        nc.sync.dma_start(
                            out=w2pre_t[ic][:],
                            in_=ff2wT[ic * 128:(ic + 1) * 128, 0:512])
        if DEBUG:
            nc.sync.dma_start(out=dbg_qt[:], in_=QTp[0][:])
            nc.sync.dma_start(out=dbg_kt[:], in_=KTp[0][:])
            nc.sync.dma_start(out=dbg_ctx[:], in_=ctxT[:])
            for q in range(RQ):
                nc.sync.dma_start(out=dbg_rs[q * 128:(q + 1) * 128, :],
                                  in_=rs_out[q][:])
        at.release()

        # ---- phase C: LN2 (q0-2 first, q3 after its RS lands) + FF.
        # FF1 runs in two passes so cols for q0-2 compute during RS chunk 3.
        with tc.tile_pool(name="ff", bufs=1) as ff, \
             tc.tile_pool(name="ffs", bufs=2) as ffs, \
             tc.tile_pool(name="w1p", bufs=16) as w1p, \
             tc.tile_pool(name="w2p", bufs=8) as w2p, \
             tc.tile_pool(name="ps_f1", bufs=2, space="PSUM") as ps_f1, \
             tc.tile_pool(name="ps_f2", bufs=1, space="PSUM") as ps_f2:

            def emit_ln2(q):
                rl = ffs.tile([128, H], BF16, tag="rsld")
                nc.sync.dma_start(out=rl[:], in_=rs_out[q][:])
                x2 = x2_tiles[q]
                nc.vector.tensor_add(x2[:], rl[:], xres_tiles[q][:])
                nc.vector.tensor_add(x2[:], x2[:], ob_bc[:])
                ns = ffs.tile([128, 1], F32, tag="negsum2")
                nc.vector.reduce_sum(out=ns[:], in_=x2[:], axis=AXX,
                                     negate=True)
                nm = ffs.tile([128, 1], F32, tag="negmean2")
                nc.scalar.mul(nm[:], ns[:], 1.0 / H)
                xn2 = ffs.tile([128, H], F32, tag="xn2q")
                nc.scalar.activation(xn2[:], x2[:], AF.Identity, bias=nm[:])
                for hc in range(HC):
                    ps = ps_f1.tile([128, 128], F32, tag="tp", bufs=1)
                    nc.tensor.transpose(ps[:], xn2[:, hc * 128:(hc + 1) * 128],
                                        ident_f[:])
                    nc.vector.tensor_copy(xn2T[hc][:, q * 128:(q + 1) * 128],
                                          ps[:])

            for q in range(3):
                emit_ln2(q)

            hT = [ff.tile([128, SM], BF16, tag=f"hT{i}", name=f"hT{i}")
                  for i in range(32)]
            for ib in range(8):
                if ib < 4:
                    w1t = w1pre_t[ib]
                else:
                    w1t = []
                    for hc in range(HC):
                        t = w1p.tile([128, 512], BF16, tag="w1",
                                     name=f"w1_{ib}_{hc}")
                        nc.sync.dma_start(
                            out=t[:],
                            in_=ff1wT[hc * 128:(hc + 1) * 128,
                                      ib * 512:(ib + 1) * 512])
                        w1t.append(t)
                for sub in range(4):
                    it = ib * 4 + sub
                    ps = ps_f1.tile([128, 384], F32, tag="f1")
                    for hc in range(HC):
                        nc.tensor.matmul(ps[:],
                                         w1t[hc][:, sub * 128:(sub + 1) * 128],
                                         xn2T[hc][:, 0:384], start=(hc == 0),
                                         stop=(hc == 7))
                    nc.scalar.activation(hT[it][:, 0:384], ps[:], AF.Relu,
                                         bias=ffb1_pp[:, it:it + 1])

            emit_ln2(3)
            for ib in range(8):
                if ib < 4:
                    w1t = w1pre_t[ib]
                else:
                    w1t = []
                    for hc in range(HC):
                        t = w1p.tile([128, 512], BF16, tag="w1",
                                     name=f"w1b_{ib}_{hc}")
                        nc.sync.dma_start(
                            out=t[:],
                            in_=ff1wT[hc * 128:(hc + 1) * 128,
                                      ib * 512:(ib + 1) * 512])
                        w1t.append(t)
                for sub in range(4):
                    it = ib * 4 + sub
                    ps = ps_f1.tile([128, 128], F32, tag="f1b", bufs=1)
                    for hc in range(HC):
                        nc.tensor.matmul(ps[:],
                                         w1t[hc][:, sub * 128:(sub + 1) * 128],
                                         xn2T[hc][:, 384:512], start=(hc == 0),
                                         stop=(hc == 7))
                    nc.scalar.activation(hT[it][:, 384:512], ps[:], AF.Relu,
                                         bias=ffb1_pp[:, it:it + 1])

            y_sb = [ff.tile([128, H], F32, tag=f"y{i}", name=f"ysb{i}")
                    for i in range(4)]
            for hf in range(2):
                yps = [ps_f2.tile([128, 512], F32, name=f"yps{hf}_{i}",
                                  tag=f"yps{i}", bufs=1) for i in range(4)]
                for ic in range(32):
                    if hf == 0 and ic < 16:
                        w2t = w2pre_t[ic]
                    else:
                        w2t = w2p.tile([128, 512], BF16, tag="w2")
                        nc.sync.dma_start(
                            out=w2t[:],
                            in_=ff2wT[ic * 128:(ic + 1) * 128,
                                      hf * 512:(hf + 1) * 512])
                    for s4 in range(4):
                        nc.tensor.matmul(yps[s4][:],
                                         hT[ic][:, s4 * 128:(s4 + 1) * 128],
                                         w2t[:], start=(ic == 0), stop=(ic == 31),
                                         skip_group_check=True)
                for s4 in range(4):
                    sl = slice(hf * 512, (hf + 1) * 512)
                    nc.vector.tensor_add(y_sb[s4][:, sl], yps[s4][:],
                                         x2_tiles[s4][:, sl])
                    nc.vector.tensor_add(y_sb[s4][:, sl], y_sb[s4][:, sl],
                                         ffb2_bc[:, sl])
                    nc.sync.dma_start(out=y[s4 * 128:(s4 + 1) * 128, sl],
                                      in_=y_sb[s4][:, sl])

        w1pre.release()
        ffpre.release()
        dram.release()
        cst.release()

    nc.compile()
    return nc


def make_in_maps(inputs):
    bf16 = ml_dtypes.bfloat16
    f = lambda a: np.asarray(a, dtype=np.float32)
    x = f(inputs["x"])
    anw, anb = f(inputs["an_w"]), f(inputs["an_b"])
    fnw, fnb = f(inputs["fn_w"]), f(inputs["fn_b"])
    # fold LN1 affine into qkv weights/biases (exact algebra)
    q_w, k_w, v_w = f(inputs["q_w"]), f(inputs["k_w"]), f(inputs["v_w"])
    qkv_w = [w * anw[None, :] for w in (q_w, k_w, v_w)]
    qkv_b = [f(inputs[n]) + w0 @ anb
             for n, w0 in (("q_b", q_w), ("k_b", k_w), ("v_b", v_w))]
    o_w = f(inputs["o_w"])
    # fold LN2 affine into ff1
    ff1_w = f(inputs["ff1_w"]) * fnw[None, :]
    ff1_b = f(inputs["ff1_b"]) + f(inputs["ff1_w"]) @ fnb
    ff2_w = f(inputs["ff2_w"])

    xT = np.ascontiguousarray(x.T.astype(bf16))
    ff1wT = np.ascontiguousarray(ff1_w.T.astype(bf16))
    ff2wT = np.ascontiguousarray(ff2_w.T.astype(bf16))
    ff1b_t = np.ascontiguousarray(ff1_b.reshape(32, 128))
    row = lambda a: np.ascontiguousarray(a.reshape(1, -1))
    in_maps = []
    for m in range(NC):
        dm = slice(m * DM, (m + 1) * DM)
        wqkvT = np.concatenate([w[dm].T for w in qkv_w], axis=1)
        nrw = -wqkvT.sum(axis=0, keepdims=True) / H
        bqkv = np.concatenate([b[dm] for b in qkv_b]).reshape(1, -1)
        # rows owned by core m: for each RS chunk q, rows q*1024+m*128 ..+128
        own = np.concatenate([x[q * 1024 + m * 128: q * 1024 + (m + 1) * 128]
                              for q in range(RQ)], axis=0)
        in_maps.append({
            "xT": xT,
            "wqkvT": np.ascontiguousarray(wqkvT.astype(bf16)),
            "nrw": np.ascontiguousarray(nrw.astype(np.float32)),
            "bqkv": np.ascontiguousarray(bqkv.astype(np.float32)),
            "owT": np.ascontiguousarray(o_w[:, dm].T.astype(bf16)),
            "ob": row(f(inputs["o_b"])),
            "ff1wT": ff1wT, "ff1b": ff1b_t,
            "ff2wT": ff2wT, "ffb2": row(f(inputs["ff2_b"])),
            "x_res": np.ascontiguousarray(own),
        })
    return in_maps


def assemble(results):
    y = np.empty((S, H), dtype=np.float32)
    for m in range(NC):
        ym = results[m]["y"]
        for q in range(RQ):
            y[q * 1024 + m * 128: q * 1024 + (m + 1) * 128] = \
                ym[q * 128:(q + 1) * 128]
    return y


def kernel(**inputs) -> np.ndarray:
    from concourse.bass_utils import run_bass_kernel_spmd
    if "nc" not in _CACHE:
        _CACHE["nc"] = build_nc()
    nc = _CACHE["nc"]
    in_maps = make_in_maps(inputs)
    res = run_bass_kernel_spmd(nc, in_maps, core_ids=list(range(NC)))
    return assemble(res.results)
